# revision 16
# baseline (speedup 1.0000x reference)
"""TT-adapter linear kernel for TRN2, data-parallel over batch on 8 NeuronCores.

Math: out = x @ W.T + b + ALPHA * TT(x).  TT is linear in x, so the module
collapses to a single matmul with a merged weight folded on host:

    Wc = W + ALPHA * T          (T = TT-matrix reconstruction, 1024x1024)
    out = x @ Wc.T + b

The 34 GFLOP batched matmul runs on device in bf16 (f32 PSUM accumulation),
one batch element per NeuronCore, no collectives.  Raw bacc (manual
semaphores).  PE floor is 256 MMs x 216 ns = 55.3 us.

Measured DMA behavior that shapes the schedule: ONE HWDGE queue, in-order,
~400 GB/s streaming but ~0.45 us FIXED cost per DMA, and a DMA's completion
sem reaches +16 only 0.3-1.4 us after its data lands (16 per-engine incs
straggle).  Two queues split bandwidth without priority (bad).  So: fewest
possible DMAs, single SP queue, strictly in need-order, each granule sized
to what the PE staircase consumes per step.  The PE must also stay
continuously busy from the preamble until real data arrives (~3.4 us), else
the HAM clock-gate re-throttles and the first ~16 real MMs run at half rate.

Host layouts (per core, P=128 partitions, contraction dim on partitions):
    wxs0 bf16 [8, 128, 1544]  [d, p, 0:1024]    = Wc[:, 128d+p]  (all o)
                              [d, p, 1024:1536] = x[b, 0:512, 128d+p] (sc=0)
                              [0, p, 1536+oo]   = b[128oo+p]  (bias, bf16)
    xs   bf16 [3, 128, 8, 512] xs[i, p, d, j] = x[b, 512(i+1)+j, 128d+p]
    out  bf16 [8, 128, 2048]   out[oo, p, s]  = result[b, s, 128oo+p]

Schedule per core (group idx = 8*sc + o; bank = o; all MMs N=512):
  SP:  8 merged (w_d|x0_d|bias) DMAs in d order -- ONE gate per staircase
       step -- then sc=1 x in two halves (d0-3, d4-7), sc=2, sc=3; then
       out-DMAs idx=0..30 gated on evictions; final wait on slot sems.
  PE:  16 HAM-warm-up matmuls (~3.4 us, keeps the clock-gate warm until
       the first gate passes); phase 1 = sc=0 strip (o=0..7) d-outer
       staircase across all 8 PSUM banks; phase 2 = sc=1..3 strips,
       d-inner per group, each group's gate waits hoisted before the
       previous group's last MM so the NX resolves them while PE streams.
  ACT: dummy 8-col activate (hoists the lazy 1.3 us ACT_TABLE_LOAD into
       the preamble), 32 evictions (PSUM -> SBUF bf16 + bias add), last
       group's out-DMA ships from ACT directly (skips the SP sem hop).
"""

import numpy as np
import ml_dtypes
from contextlib import ExitStack

import concourse.bass as bass  # noqa: F401
import concourse.mybir as mybir
from concourse import bacc
from concourse.bass_utils import run_bass_kernel_spmd

ALPHA = 16.0
B, S, D = 8, 2048, 1024
P = 128
DO = D // P          # 8 contraction tiles
OO = D // P          # 8 output tiles
SCH = 512
NS = S // SCH        # 4 s-chunks
NG = OO * NS         # 32 groups
NBANK = 8
NSLOT = 4
WXC = D + SCH + 8    # merged per-d row: 1024 w | 512 x(sc0) | 8 bias
XOFF = D
BOFF = D + SCH

_NC = None


def _build_nc():
    nc = bacc.Bacc("TRN2", target_bir_lowering=False, debug=False)
    wxs0 = nc.declare_dram_parameter("wxs0", [DO, P, WXC], mybir.dt.bfloat16, isOutput=False)
    xs = nc.declare_dram_parameter("xs", [P, NS - 1, DO, SCH], mybir.dt.bfloat16, isOutput=False)
    out = nc.declare_dram_parameter("out", [OO, P, S], mybir.dt.bfloat16, isOutput=True)

    with ExitStack() as ctx:
        block = ctx.enter_context(nc.Block(no_gpsimd_drain=True))
        # One sem per gating granule (HWDGE completions are unordered across
        # DMAs; each DMA incs its sem by 16, one per SDMA engine).
        s_wx = [ctx.enter_context(nc.semaphore(f"s_wx{d}")) for d in range(DO)]
        s_x1 = ctx.enter_context(nc.semaphore("s_x1"))     # xs[0]  (sc=1)
        s_x23 = ctx.enter_context(nc.semaphore("s_x23"))   # xs[1:] (sc=2,3)
        s_mm = ctx.enter_context(nc.semaphore("s_mm"))
        s_ev = ctx.enter_context(nc.semaphore("s_ev"))
        s_slot = [ctx.enter_context(nc.semaphore(f"s_slot{k}")) for k in range(NSLOT)]

        wx_sb = ctx.enter_context(nc.sbuf_tensor("wx_sb", [P, DO, WXC], mybir.dt.bfloat16))
        xb_sb = ctx.enter_context(nc.sbuf_tensor("xb_sb", [P, NS - 1, DO, SCH], mybir.dt.bfloat16))
        ot_sb = ctx.enter_context(nc.sbuf_tensor("ot_sb", [P, NSLOT, SCH], mybir.dt.bfloat16))
        ps = [ctx.enter_context(nc.psum_tensor(f"ps{b}", [P, SCH], mybir.dt.float32))
              for b in range(NBANK)]

        def wsl(o, d):
            return wx_sb[:, d, o * P:(o + 1) * P]

        def xsl(sc, d):
            if sc == 0:
                return wx_sb[:, d, XOFF:XOFF + SCH]
            return xb_sb[:, sc - 1, d, :]

        def bias_ap(o):
            return wx_sb[:, 0, BOFF + o:BOFF + o + 1]

        @block.sync
        def _(sync: bass.BassEngine):
            # strict need-order, one DMA per staircase step
            for d in range(DO):
                sync.dma_start(out=wx_sb[:, d, :], in_=wxs0[d]).then_inc(s_wx[d], 16)
            sync.dma_start(out=xb_sb[:, 0, :, :], in_=xs[:, 0, :, :]).then_inc(s_x1, 16)
            sync.dma_start(out=xb_sb[:, 1:, :, :], in_=xs[:, 1:, :, :]).then_inc(s_x23, 16)
            for g in range(NG - 1):
                o, sc = g % OO, g // OO
                sync.wait_ge(s_ev, g + 1)
                sync.dma_start(
                    out=out[o, :, sc * SCH:(sc + 1) * SCH],
                    in_=ot_sb[:, g % NSLOT, :],
                ).then_inc(s_slot[g % NSLOT], 16)
            for k in range(NSLOT):
                sync.wait_ge(s_slot[k], 16 * (NG // NSLOT))

        @block.tensor
        def _(tensor: bass.BassEngine):
            # HAM warm-up: ~3.4us of continuous dummy matmuls so the PE
            # clock-gate reaches 8/8 and STAYS there until the first real
            # gate passes; results discarded (bank 0 restarts, start=True).
            for _ in range(16):
                tensor.matmul(
                    ps[0][:, 0:256],
                    wx_sb[:, 0, 0:P],
                    wx_sb[:, 0, XOFF:XOFF + 256],
                    start=True,
                    stop=True,
                )
            # phase 1: sc=0 strip, d-outer staircase over banks 0..7 (=o).
            # The NEXT step's gate wait is hoisted before each step's last
            # MM so the NX resolves it while the PE streams (d+1's DMA sem
            # never depends on the PE, so this cannot deadlock).
            for d in range(DO):
                if d == 0:
                    tensor.wait_ge(s_wx[0], 16)
                for o in range(OO):
                    if o == OO - 1:
                        if d < DO - 1:
                            tensor.wait_ge(s_wx[d + 1], 16)
                        else:
                            # phase-2 g=8 gates: ev(0) completes ~0.2us
                            # before this MM would issue (s_mm(1) fired 7
                            # MMs ago), so this wait is already satisfied
                            tensor.wait_ge(s_x1, 16)
                            tensor.wait_ge(s_ev, 1)
                    mmi = tensor.matmul(
                        ps[o][:, :],
                        wsl(o, d),
                        xsl(0, d),
                        start=(d == 0),
                        stop=(d == DO - 1),
                    )
                    if d == DO - 1:
                        # d=7 octet runs in group order 0..7 -> s_mm incs
                        # arrive in the order the evictions expect
                        mmi.then_inc(s_mm, 1)
            # phase 2: sc=1..3 strips, d-inner per group.  Group g's gate
            # waits are emitted before the previous group's LAST matmul
            # (the waited-on eviction g-8 completed ~12us earlier, so this
            # only saves latency, never blocks the stream).
            for g in range(NBANK, NG):
                o, sc = g % OO, g // OO
                for d in range(DO):
                    if d == DO - 1 and g + 1 < NG:
                        no, nsc = (g + 1) % OO, (g + 1) // OO
                        if no == 0 and nsc == 2:
                            tensor.wait_ge(s_x23, 16)
                        tensor.wait_ge(s_ev, g + 1 - NBANK + 1)
                    mmi = tensor.matmul(
                        ps[o][:, :],
                        wsl(o, d),
                        xsl(sc, d),
                        start=(d == 0),
                        stop=(d == DO - 1),
                    )
                    if d == DO - 1:
                        mmi.then_inc(s_mm, 1)

        @block.scalar
        def _(scalar: bass.BassEngine):
            # dummy 8-col activate: pulls the lazy ACT_TABLE_LOAD into the
            # preamble window (it otherwise delays the first real eviction
            # by ~1.3us).  Reads garbage; slot 0 is fully overwritten by
            # eviction 0 before any out-DMA reads it.
            scalar.add(ot_sb[:, 0, 0:8], ot_sb[:, 1, 0:8], bias_ap(0))
            for g in range(NG):
                o, sc = g % OO, g // OO
                if g == 0:
                    scalar.wait_ge(s_wx[0], 16)   # bias rides in wxs0[0]
                scalar.wait_ge(s_mm, g + 1)
                if g >= NSLOT:
                    scalar.wait_ge(s_slot[g % NSLOT], 16 * (g // NSLOT))
                scalar.add(
                    ot_sb[:, g % NSLOT, :], ps[o][:, :], bias_ap(o)
                ).then_inc(s_ev, 1)
                if g == NG - 1:
                    # last output ships from ACT (also HWDGE, its own queue):
                    # skips the SP semaphore hop on the critical tail
                    scalar.dma_start(
                        out=out[o, :, sc * SCH:(sc + 1) * SCH],
                        in_=ot_sb[:, g % NSLOT, :],
                    ).then_inc(s_slot[g % NSLOT], 16)

    nc.compile()
    return nc


def _get_nc():
    global _NC
    if _NC is None:
        _NC = _build_nc()
    return _NC


def _merged_weight_T(W, b, core0, core1, core2, core3, core4, core5):
    f8 = np.float64
    A = core0[0].astype(f8)
    Bm = np.einsum('ap,pbq->abq', A, core1.astype(f8))
    C = np.einsum('abq,qcr->abcr', Bm, core2.astype(f8))
    Phi = C.transpose(2, 1, 0, 3).reshape(D, 8)
    Dn = np.einsum('paq,qbr->pabr', core3.astype(f8), core4.astype(f8))
    E = np.einsum('pabq,qc->pabc', Dn, core5[:, :, 0].astype(f8))
    Psi = E.reshape(8, D)
    WcT = W.T.astype(f8) + ALPHA * (Phi @ Psi)
    return WcT.astype(np.float32)


def _prep_in_maps(x, W, b, core0, core1, core2, core3, core4, core5):
    WcT = _merged_weight_T(W, b, core0, core1, core2, core3, core4, core5)
    wt16 = WcT.reshape(DO, P, D).astype(ml_dtypes.bfloat16)
    bias_pad = np.zeros((DO, P, 8), dtype=ml_dtypes.bfloat16)
    bias_pad[0] = b.reshape(OO, P).T.astype(ml_dtypes.bfloat16)
    in_maps = []
    for bb in range(B):
        xt = x[bb].T.reshape(DO, P, NS, SCH)          # [d, p, sc, j]
        x0 = xt[:, :, 0, :].astype(ml_dtypes.bfloat16)
        wxs0 = np.ascontiguousarray(
            np.concatenate([wt16, x0, bias_pad], axis=2))
        xsb = np.ascontiguousarray(
            xt[:, :, 1:, :].transpose(1, 2, 0, 3)     # [p, sc-1, d, j]
        ).astype(ml_dtypes.bfloat16)
        in_maps.append({"wxs0": wxs0, "xs": xsb})
    return in_maps


def _gather(results):
    outs = []
    for bb in range(B):
        o = np.asarray(results[bb]["out"]).astype(np.float32)
        outs.append(o.transpose(2, 0, 1).reshape(S, D))
    return np.ascontiguousarray(np.stack(outs))


def run(inputs, **spmd_kwargs):
    inputs = {k: np.asarray(v) for k, v in inputs.items()}
    in_maps = _prep_in_maps(**inputs)
    nc = _get_nc()
    res = run_bass_kernel_spmd(nc, in_maps, core_ids=list(range(B)), **spmd_kwargs)
    return _gather(res.results), res


def kernel(x, W, b, core0, core1, core2, core3, core4, core5):
    out, _ = run(dict(x=x, W=W, b=b, core0=core0, core1=core1, core2=core2,
                      core3=core3, core4=core4, core5=core5))
    return out
